# revision 3
# baseline (speedup 1.0000x reference)
"""Trainium2 Bass kernel for nn_ButterflyNetwork (self-contained).

Strategy:
- Pure data parallelism: batch 4096 -> 8 cores x 512 columns, identical program.
- All rotations composed on the host into block-diagonal 16x16 matrices; the
  in-rotation for non-activated rows is folded THROUGH the out-rotation into a
  single 128x128 matrix C per tile (y_nonact never materializes). Input scaling
  is folded into the first-consumer matrix columns.
- Canonical data rows live in a per-core DRAM arena [5120, 512] fp16 (row-major,
  1KB rows). Per module: one indirect-DMA gather of 1024 rows -> SBUF x-tiles,
  matmuls (fp16 weights, fp32 PSUM), smoothed-ReLU on ACT+DVE, z rows
  (live-only) indirect-scattered back, activation rows stored contiguously.
- Output = module 7's activation rows, cast fp16->fp32 on the final DMA.
"""
import numpy as np

# ---- problem constants (hardcoded per contract) ----
COLB = 16
IN_W = 1024
OUT_W = 512
DEPTH = 8
IN_L = 4
OUT_L = 4
ACT = 8
BLOCKS = 64
CURV = 1.0
GROW = BLOCKS * ACT
TOTAL = IN_W + DEPTH * GROW  # 5120
BATCH = 4096
N_CORES = 8
BL = BATCH // N_CORES  # 512
W = BLOCKS * COLB  # 1024
NTILES = 8
NBANKS = 4

LAST_EXEC_NS = None  # set when profiling enabled


# ---------------------------------------------------------------- host math
def _rotate(x, ang, stride):
    W_, B = x.shape
    xr = x.reshape(W_ // (2 * stride), 2, stride, B)
    a = ang.reshape(W_ // (2 * stride), stride)[:, :, None]
    cth, sth = np.cos(a), np.sin(a)
    lo, hi = xr[:, 0], xr[:, 1]
    return np.stack([cth * lo + sth * hi, -sth * lo + cth * hi], axis=1).reshape(W_, B)


def _module_rot_matrices(ang):
    I = np.eye(W)
    Min = I.copy()
    for l in range(IN_L):
        Min = _rotate(Min, ang[l], 2 ** (l % 4))
    Mout = I.copy()
    for l in range(OUT_L):
        Mout = _rotate(Mout, ang[IN_L + l], 2 ** ((IN_L + l) % 4))
    Min_b = np.stack([Min[16 * b:16 * b + 16, 16 * b:16 * b + 16] for b in range(BLOCKS)])
    Mout_b = np.stack([Mout[16 * b:16 * b + 16, 16 * b:16 * b + 16] for b in range(BLOCKS)])
    return Min_b, Mout_b


class _Consts:
    pass


def _build_constants(angles, biases, indices_in, scales):
    angles = np.asarray(angles, np.float64)
    biases = np.asarray(biases, np.float64)
    scales = np.asarray(scales, np.float64)
    idx = np.asarray(indices_in, np.int64)

    c = _Consts()
    c.Min, c.Mout = [], []
    for j in range(DEPTH):
        Min_b, Mout_b = _module_rot_matrices(angles[j])
        c.Min.append(Min_b)
        c.Mout.append(Mout_b)

    read_by = [set(idx[i].tolist()) for i in range(DEPTH)]
    c.z_live = []
    for j in range(DEPTH):
        if j == DEPTH - 1:
            c.z_live.append(np.zeros(W, bool))
            continue
        live = np.zeros(W, bool)
        for r in range(W):
            row = idx[j][r]
            for i in range(j + 1, DEPTH):
                if row in read_by[i]:
                    live[r] = True
                    break
        c.z_live.append(live)

    # consumer module of each z row (or -1 if dead)
    z_consumer = np.full((DEPTH, W), -1, np.int64)
    for j in range(DEPTH - 1):
        for r in range(W):
            row = int(idx[j][r])
            for i in range(j + 1, DEPTH):
                if row in read_by[i]:
                    z_consumer[j, r] = i
                    break
    c.z_order, c.z_nlive, c.z_nhot = [], [], []
    for j in range(DEPTH):
        orders, nlives, nhots = [], [], []
        for T in range(NTILES):
            loc = np.arange(128)
            cons = z_consumer[j, 128 * T + loc]
            hot = loc[cons == j + 1]
            cold = loc[cons > j + 1]
            dead = loc[cons == -1]
            orders.append(np.concatenate([hot, cold, dead]))
            nlives.append(int(len(hot) + len(cold)))
            nhots.append(int(len(hot)))
        c.z_order.append(orders)
        c.z_nlive.append(nlives)
        c.z_nhot.append(nhots)

    first_read = {}
    for j in range(DEPTH):
        for row in idx[j]:
            r = int(row)
            if r not in first_read:
                first_read[r] = j

    def src_factor(row, j):
        return scales[row] if (row < IN_W and first_read.get(int(row)) == j) else 1.0

    # factor per (module, tile, k): vectorized fold
    fac = np.ones((DEPTH, NTILES, 128))
    for j in range(DEPTH):
        for T in range(NTILES):
            for k in range(128):
                fac[j, T, k] = src_factor(idx[j][128 * T + k], j)

    c.W_act = np.zeros((DEPTH, NBANKS, 2, 128, 64))
    for j in range(DEPTH):
        for a in range(NBANKS):
            for h in range(2):
                T = 2 * a + h
                for m in range(64):
                    r_act = 128 * a + 64 * h + m
                    b = r_act // ACT
                    pos = r_act % ACT
                    bloc = b - 8 * T
                    ks = 16 * bloc + np.arange(16)
                    c.W_act[j, a, h, ks, m] = c.Min[j][b][pos, :] * fac[j, T, ks]

    c.C = np.zeros((DEPTH, NTILES, 128, 128))
    c.D = np.zeros((DEPTH, NTILES, 64, 128))
    for j in range(DEPTH - 1):
        Min_b, Mout_b = c.Min[j], c.Mout[j]
        # per block: composed nonact transform [16 out, 16 in]
        comp = np.einsum("bpk,bki->bpi", Mout_b[:, :, ACT:], Min_b[:, ACT:, :])
        for T in range(NTILES):
            order = c.z_order[j][T]
            for m_idx in range(128):
                r = 128 * T + order[m_idx]
                b = r // 16
                pos = r % 16
                bloc = b - 8 * T
                ks = 16 * bloc + np.arange(16)
                c.C[j, T, ks, m_idx] = comp[b][pos, :] * fac[j, T, ks]
                c.D[j, T, 8 * bloc + np.arange(ACT), m_idx] = Mout_b[b][pos, :ACT]

    c.bias = biases.reshape(DEPTH, NBANKS, 128)

    # consumer-ordered arena: slot s = 1024*j + position-in-idx_j. For each
    # produced row (init row / z row / act row) find its consuming module's slot.
    OOB = 1 << 20
    pos_in = [dict() for _ in range(DEPTH)]
    for j in range(DEPTH):
        for p, row in enumerate(idx[j]):
            pos_in[j][int(row)] = p

    def consumer_slot(row, after):
        row = int(row)
        for i in range(after, DEPTH):
            p = pos_in[i].get(row)
            if p is not None:
                return 1024 * i + p
        return OOB

    # init scatter offsets: input tile T row p (row id 128T+p)
    c.ioff = np.full((NTILES, 128), OOB, np.int32)
    for T in range(NTILES):
        for p in range(128):
            c.ioff[T, p] = consumer_slot(128 * T + p, 0)
    # x0-direct: xin row feeding module-0 x tile T partition p
    c.x0off = np.zeros((128, NTILES), np.int32)
    for T in range(NTILES):
        c.x0off[:, T] = idx[0][128 * T + np.arange(128)]
    # z scatter offsets (live-first order; only [0:nlive] used)
    c.soff = np.full((DEPTH, NTILES, 128), OOB, np.int32)
    for j in range(DEPTH - 1):
        for T in range(NTILES):
            for m, rl in enumerate(c.z_order[j][T]):
                r = 128 * T + int(rl)
                if c.z_live[j][r]:
                    c.soff[j, T, m] = consumer_slot(idx[j][r], j + 1)
    # act scatter offsets: bank a row q = act row 128a+q = arena row IN_W+512j+128a+q
    c.aoff = np.full((DEPTH - 1, NBANKS, 128), OOB, np.int32)
    for j in range(DEPTH - 1):
        for a in range(NBANKS):
            for q in range(128):
                c.aoff[j, a, q] = consumer_slot(IN_W + GROW * j + 128 * a + q, j + 1)
    return c


# ------------------------------------------------- walrus sync-wait workaround
def _split_sync_waits(nc, limit=1):
    """This container's walrus build rejects >1 semaphore wait per instruction
    ("Too many sync wait commands"). Move excess waits onto NoOps placed just
    before the instruction on the same engine queue — the sequencer stalls at
    each NoOp's wait, so ordering semantics are identical."""
    import concourse.mybir as mybir

    seq = [0]
    for f in nc.m.functions:
        for bb in f.blocks:
            insts = bb.instructions
            newlist = []
            changed = False
            for inst in insts:
                si = getattr(inst, "sync_info", None)
                waits = list(si.on_wait) if si is not None else []
                if len(waits) > limit:
                    changed = True
                    for w in waits[:-limit]:
                        nop = mybir.InstNoOp(
                            name=f"waitsplit-{seq[0]}", ins=[], outs=[])
                        seq[0] += 1
                        nop.engine = inst.engine
                        nop.sync_info = mybir.SyncInfo(on_wait=[w], on_update=[])
                        newlist.append(nop)
                    inst.sync_info = mybir.SyncInfo(
                        on_wait=waits[-limit:], on_update=list(si.on_update))
                newlist.append(inst)
            if changed:
                bb.instructions = newlist


def _strip_scatter_dma_waits(nc, scatter_names):
    """All arena scatters write disjoint slots (each consumer slot is written
    exactly once), so scatter->scatter WAW and scatter-vs-gather WAR deps that
    Tile emits conservatively (dynamic APs) are false. True input deps of the
    scatters (offset tables, source tiles) are routed through compute-engine
    sems, so it is safe to drop DMA-lane (DMASW*/DMAHW*) waits on them."""
    import concourse.mybir as mybir

    for f in nc.m.functions:
        for bb in f.blocks:
            for inst in bb.instructions:
                if inst.name not in scatter_names:
                    continue
                si = getattr(inst, "sync_info", None)
                if si is None:
                    continue
                keep = [w for w in si.on_wait
                        if not (w.ant_name or "").startswith(("DMASW", "DMAHW"))]
                if len(keep) != len(si.on_wait):
                    inst.sync_info = mybir.SyncInfo(
                        on_wait=keep, on_update=list(si.on_update))


# ---------------------------------------------------------------- bass build
def _build_bass(c, repeat=1, knobs=None):
    import concourse.bass as bass
    import concourse.mybir as mybir
    import concourse.tile as tile
    from contextlib import ExitStack

    knobs = knobs or {}
    kn_nq = knobs.get("nq", 4)
    kn_actsq = knobs.get("actsq", "dve")
    kn_zsplit = knobs.get("zsplit", True)
    kn_scatter = knobs.get("scatter", "indirect")
    kn_actpath = knobs.get("actpath", True)
    kn_zmm = knobs.get("zmm", True)
    f16, f32, i32 = mybir.dt.float16, mybir.dt.float32, mybir.dt.int32
    AF = mybir.ActivationFunctionType
    OP = mybir.AluOpType

    nc = bass.Bass(trn_type="TRN2", num_swdge_queues=max(1, kn_nq))
    xin = nc.dram_tensor("xin", [IN_W, BL], f32, kind="ExternalInput")
    out = nc.dram_tensor("out", [OUT_W, BL], f32, kind="ExternalOutput")

    # inline constants
    wact_np = np.zeros((128, DEPTH * NBANKS * 2 * 64), np.float16)
    for j in range(DEPTH):
        for a in range(NBANKS):
            for h in range(2):
                col = ((j * NBANKS + a) * 2 + h) * 64
                wact_np[:, col:col + 64] = c.W_act[j, a, h].astype(np.float16)
    cmat_np = np.zeros((128, (DEPTH - 1) * NTILES * 128), np.float16)
    dmat_np = np.zeros((128, (DEPTH - 1) * NTILES * 128), np.float16)
    for j in range(DEPTH - 1):
        for T in range(NTILES):
            col = (j * NTILES + T) * 128
            cmat_np[:, col:col + 128] = c.C[j, T].astype(np.float16)
            po = 64 * (T % 2)
            dmat_np[po:po + 64, col:col + 128] = c.D[j, T].astype(np.float16)
    # last column of bias_np = 0.25 constant (Sqrt bias)
    bias_np = np.zeros((128, DEPTH * NBANKS + 1), np.float32)
    bias_np[:, DEPTH * NBANKS] = 0.25
    hbias_np = np.zeros((128, DEPTH * NBANKS), np.float32)
    for j in range(DEPTH):
        for a in range(NBANKS):
            bias_np[:, j * NBANKS + a] = c.bias[j, a].astype(np.float32)
            hbias_np[:, j * NBANKS + a] = (0.5 * c.bias[j, a]).astype(np.float32)
    soff_np = np.zeros((128, DEPTH * NTILES), np.int32)
    soffc_np = np.zeros((128, DEPTH * NTILES), np.int32)
    aoff_np = np.zeros((128, (DEPTH - 1) * NBANKS), np.int32)
    ioff_np = np.zeros((128, NTILES), np.int32)
    for j in range(DEPTH - 1):
        for T in range(NTILES):
            soff_np[:, j * NTILES + T] = c.soff[j, T]
            nh, nlv = c.z_nhot[j][T], c.z_nlive[j][T]
            soffc_np[0:nlv - nh, j * NTILES + T] = c.soff[j, T][nh:nlv]
        for a in range(NBANKS):
            aoff_np[:, j * NBANKS + a] = c.aoff[j, a]
    for T in range(NTILES):
        ioff_np[:, T] = c.ioff[T]
    x0off_np = c.x0off.astype(np.int32)

    wact_t = nc.inline_tensor(wact_np, name="wact")
    cmat_t = nc.inline_tensor(cmat_np, name="cmat")
    dmat_t = nc.inline_tensor(dmat_np, name="dmat")
    bias_t = nc.inline_tensor(bias_np, name="biast")
    hbias_t = nc.inline_tensor(hbias_np, name="hbiast")
    soff_t = nc.inline_tensor(soff_np, name="sofft")
    soffc_t = nc.inline_tensor(soffc_np, name="soffct")
    aoff_t = nc.inline_tensor(aoff_np, name="aofft")
    ioff_t = nc.inline_tensor(ioff_np, name="iofft")
    x0off_t = nc.inline_tensor(x0off_np, name="x0offt")

    with tile.TileContext(nc) as tc, ExitStack() as ctx:
        const = ctx.enter_context(tc.tile_pool(name="const", bufs=1))
        xpool = ctx.enter_context(tc.tile_pool(name="x", bufs=2))
        apool = ctx.enter_context(tc.tile_pool(name="actp", bufs=2))
        zpool = ctx.enter_context(tc.tile_pool(name="z", bufs=2))
        pspool = ctx.enter_context(tc.tile_pool(name="ps", bufs=8, space="PSUM"))
        dram = ctx.enter_context(tc.tile_pool(name="dram", bufs=1, space="DRAM"))

        wact_sb = const.tile([128, wact_np.shape[1]], f16, tag="wact")
        cmat_sb = const.tile([128, cmat_np.shape[1]], f16, tag="cmat")
        dmat_sb = const.tile([128, dmat_np.shape[1]], f16, tag="dmat")
        bias_sb = const.tile([128, bias_np.shape[1]], f32, tag="bias")
        hbias_sb = const.tile([128, hbias_np.shape[1]], f32, tag="hbias")
        soff_sb = const.tile([128, soff_np.shape[1]], i32, tag="soff")
        soffc_sb = const.tile([128, soffc_np.shape[1]], i32, tag="soffc")
        aoff_sb = const.tile([128, aoff_np.shape[1]], i32, tag="aoff")
        ioff_sb = const.tile([128, ioff_np.shape[1]], i32, tag="ioff")
        x0_ld = const.tile([128, NTILES], i32, tag="x0_ld")
        x0_sb = const.tile([128, NTILES], i32, tag="x0")
        soff_ld = const.tile([128, soff_np.shape[1]], i32, tag="soff_ld")
        soffc_ld = const.tile([128, soffc_np.shape[1]], i32, tag="soffc_ld")
        aoff_ld = const.tile([128, aoff_np.shape[1]], i32, tag="aoff_ld")
        ioff_ld = const.tile([128, ioff_np.shape[1]], i32, tag="ioff_ld")
        for sb_t, dr_t in [(wact_sb, wact_t), (cmat_sb, cmat_t), (dmat_sb, dmat_t),
                           (bias_sb, bias_t), (hbias_sb, hbias_t),
                           (soff_ld, soff_t), (soffc_ld, soffc_t),
                           (aoff_ld, aoff_t), (ioff_ld, ioff_t),
                           (x0_ld, x0off_t)]:
            nc.sync.dma_start(out=sb_t[:], in_=dr_t[:])
        # offset tables rerouted through DVE so scatters' true deps are
        # compute-engine sems (see _strip_scatter_dma_waits)
        nc.vector.tensor_copy(out=soff_sb[:], in_=soff_ld[:])
        nc.vector.tensor_copy(out=soffc_sb[:], in_=soffc_ld[:])
        nc.vector.tensor_copy(out=aoff_sb[:], in_=aoff_ld[:])
        nc.vector.tensor_copy(out=ioff_sb[:], in_=ioff_ld[:])
        nc.vector.tensor_copy(out=x0_sb[:], in_=x0_ld[:])

        arena = dram.tile([NTILES * 1024, BL], f16, tag="arena")
        ARENA_MAX = NTILES * 1024 - 1
        bc_reg = nc.gpsimd.to_reg(ARENA_MAX)  # shared bounds-check register
        scatter_names = set()
        _qrr = [0]

        class _QPatch:
            def __init__(self, qname):
                self.qname = qname

            def __enter__(self):
                self._orig = mybir.InstDMACopy
                qname, orig = self.qname, self._orig

                def wrapper(*a, **kw):
                    kw["queue"] = qname
                    return orig(*a, **kw)

                mybir.InstDMACopy = wrapper

            def __exit__(self, *e):
                mybir.InstDMACopy = self._orig

        def _indirect_scatter(offs_ap, src_ap):
            q = _qrr[0] % kn_nq
            _qrr[0] += 1
            qname = f"qPoolDynamic{q}" if q else "qPoolDynamic"
            with _QPatch(qname):
                bi = nc.gpsimd.indirect_dma_start(
                    out=arena[:],
                    out_offset=bass.IndirectOffsetOnAxis(ap=offs_ap, axis=0),
                    in_=src_ap, in_offset=None,
                    bounds_check=bc_reg, oob_is_err=False,
                )
            scatter_names.add(bi.ins.name)
            return bi


        for _rep in range(repeat):
          deferred_cold = []
          for j in range(DEPTH):
            # ---- gather x tiles
              xs = []
              for T in range(NTILES):
                  xt = xpool.tile([128, BL], f16, tag=f"x{T}")
                  if j == 0:
                      q = T % kn_nq
                      qname = f"qPoolDynamic{q}" if q else "qPoolDynamic"
                      with _QPatch(qname):
                          nc.gpsimd.indirect_dma_start(
                              out=xt[:], out_offset=None, in_=xin[:],
                              in_offset=bass.IndirectOffsetOnAxis(
                                  ap=x0_sb[:, T:T + 1], axis=0),
                              bounds_check=None)
                  else:
                      base_r = 1024 * j + 128 * T
                      eng = nc.sync if T % 2 == 0 else nc.scalar
                      eng.dma_start(out=xt[:], in_=arena[base_r:base_r + 128, :])
                  xs.append(xt[:])

              # ---- previous module's cold scatters (off the hot chain: their
              # conservative WAR-vs-gather deps are stripped later)
              for offs_ap, src_ap in deferred_cold:
                  _indirect_scatter(offs_ap, src_ap)
              deferred_cold = []
              # ---- act banks + activation
              aos = []
              for a in range(NBANKS):
                  ps = pspool.tile([128, BL], f32, tag="ps", space="PSUM")
                  for h in range(2):
                      wcol = ((j * NBANKS + a) * 2 + h) * 64
                      nc.tensor.matmul(
                          out=ps[64 * h:64 * h + 64, :],
                          lhsT=wact_sb[:, wcol:wcol + 64],
                          rhs=xs[2 * a + h],
                          start=True, stop=True,
                          tile_position=(0, 64 * h),
                      )
                  bcol = j * NBANKS + a
                  if not kn_actpath:
                      ao = apool.tile([128, BL], f16, tag=f"ao{a}")
                      nc.vector.tensor_copy(out=ao[:], in_=ps[:])
                      aos.append(ao)
                      continue
                  # u = 0.5*(y+b); sq = u^2 (DVE, fp16); v = sqrt(sq + 0.25) = 0.5*sqrt(pre^2+1)
                  u = apool.tile([128, BL], f16, tag="u")
                  nc.vector.tensor_scalar(out=u[:], in0=ps[:], scalar1=0.5,
                                          scalar2=hbias_sb[:, bcol:bcol + 1],
                                          op0=OP.mult, op1=OP.add)
                  sq = apool.tile([128, BL], f16, tag="sq")
                  v = apool.tile([128, BL], f16, tag="v")
                  qcol = DEPTH * NBANKS
                  if kn_actsq == "dve":
                      nc.vector.tensor_tensor(out=sq[:], in0=u[:], in1=u[:], op=OP.mult)
                      nc.scalar.activation(out=v[:], in_=sq[:], func=AF.Sqrt,
                                           bias=bias_sb[:, qcol:qcol + 1], scale=1.0)
                  else:
                      nc.scalar.activation(out=sq[:], in_=ps[:], func=AF.Square,
                                           bias=bias_sb[:, bcol:bcol + 1], scale=1.0)
                      nc.scalar.activation(out=v[:], in_=sq[:], func=AF.Sqrt,
                                           bias=bias_sb[:, qcol:qcol + 1], scale=0.25)
                  ao = apool.tile([128, BL], f16, tag=f"ao{a}")
                  nc.vector.tensor_tensor(out=ao[:], in0=u[:], in1=v[:], op=OP.add)
                  aos.append(ao)

              if j == DEPTH - 1:
                  for a in range(NBANKS):
                      nc.gpsimd.dma_start(out=out[128 * a:128 * a + 128, :], in_=aos[a][:])
                  break

              # ---- z tiles: C @ x + D @ act_out, evac live rows, scatter
              for T in range(NTILES):
                  nlv = c.z_nlive[j][T]
                  if nlv == 0:
                      continue
                  col = (j * NTILES + T) * 128
                  po = 64 * (T % 2)
                  ps = pspool.tile([128, BL], f32, tag="ps", space="PSUM")
                  if kn_zmm:
                      nc.tensor.matmul(out=ps[:], lhsT=cmat_sb[:, col:col + 128],
                                       rhs=xs[T], start=True, stop=False)
                      nc.tensor.matmul(out=ps[:], lhsT=dmat_sb[po:po + 64, col:col + 128],
                                       rhs=aos[T // 2][po:po + 64, :],
                                       start=False, stop=True, tile_position=(po, 0))
                  else:
                      nc.tensor.matmul(out=ps[:], lhsT=cmat_sb[:, col:col + 128],
                                       rhs=xs[T], start=True, stop=True)
                  zq = zpool.tile([128, BL], f16, tag=f"z{T}")
                  if T % 2 == 0:
                      nc.vector.tensor_copy(out=zq[0:nlv, :], in_=ps[0:nlv, :])
                  else:
                      nc.scalar.copy(out=zq[0:nlv, :], in_=ps[0:nlv, :])
                  scol = j * NTILES + T
                  nh = c.z_nhot[j][T]
                  if kn_scatter != "none":
                      if not kn_zsplit:
                          _indirect_scatter(soff_sb[0:nlv, scol:scol + 1], zq[0:nlv, :])
                      else:
                          if nh > 0:
                              _indirect_scatter(soff_sb[0:nh, scol:scol + 1], zq[0:nh, :])
                          if nlv > nh:
                              deferred_cold.append(
                                  (soffc_sb[0:nlv - nh, scol:scol + 1], zq[nh:nlv, :]))
              # ---- act rows -> consumer slots
              if kn_scatter != "none":
                  for a in range(NBANKS):
                      acol = j * NBANKS + a
                      _indirect_scatter(aoff_sb[:, acol:acol + 1], aos[a][:])
    _strip_scatter_dma_waits(nc, scatter_names)
    _split_sync_waits(nc)
    return nc


# ---------------------------------------------------------------- entry point
def _time_pjrt(nc, in_maps, n_runs):
    """Replicate bass2jax.run_bass_via_pjrt's multi-core path, with a timing
    loop over executions (inputs pre-uploaded; donated zero outputs re-uploaded
    outside the timed region). Returns (results, min_wall_ns_per_exec)."""
    import time
    import jax
    import jax.numpy as jnp
    from jax.sharding import Mesh, PartitionSpec
    from jax.experimental.shard_map import shard_map
    import concourse.mybir as mybir
    from concourse import bass2jax

    bass2jax.install_neuronx_cc_hook()
    n_cores = len(in_maps)
    partition_name = nc.partition_id_tensor.name if nc.partition_id_tensor else None
    in_names, out_names, out_avals, zero_outs = [], [], [], []
    for alloc in nc.m.functions[0].allocations:
        if not isinstance(alloc, mybir.MemoryLocationSet):
            continue
        name = alloc.memorylocations[0].name
        if alloc.kind == "ExternalInput":
            if name != partition_name:
                in_names.append(name)
        elif alloc.kind == "ExternalOutput":
            shape = tuple(alloc.tensor_shape)
            dtype = mybir.dt.np(alloc.dtype)
            out_names.append(name)
            out_avals.append(jax.core.ShapedArray(shape, dtype))
            zero_outs.append(np.zeros(shape, dtype))
    n_params = len(in_names)
    n_outs = len(out_avals)
    in_names_all = in_names + out_names + ([partition_name] if partition_name else [])
    donate = tuple(range(n_params, n_params + n_outs))

    def _body(*args):
        operands = list(args)
        if partition_name is not None:
            operands.append(bass2jax.partition_id_tensor())
        outs = bass2jax._bass_exec_p.bind(
            *operands,
            out_avals=tuple(out_avals),
            in_names=tuple(in_names_all),
            out_names=tuple(out_names),
            lowering_input_output_aliases=(),
            sim_require_finite=True,
            sim_require_nnan=True,
            nc=nc,
        )
        return tuple(outs)

    devices = jax.devices()[:n_cores]
    mesh = Mesh(np.asarray(devices), ("core",))
    sharded = jax.jit(
        shard_map(_body, mesh=mesh,
                  in_specs=(PartitionSpec("core"),) * (n_params + n_outs),
                  out_specs=(PartitionSpec("core"),) * n_outs, check_rep=False),
        donate_argnums=donate, keep_unused=True,
    )
    concat_in = [
        np.concatenate([np.asarray(in_maps[c][name]) for c in range(n_cores)], axis=0)
        for name in in_names
    ]
    concat_zero_shapes = [((n_cores * z.shape[0],) + z.shape[1:], z.dtype)
                          for z in zero_outs]
    from jax.sharding import NamedSharding
    shin = NamedSharding(mesh, PartitionSpec("core"))
    dev_in = [jax.device_put(x, shin) for x in concat_in]

    best = None
    out_arrs = None
    for run in range(max(1, n_runs) + 1):
        dev_zeros = [jax.device_put(jnp.zeros(s, d), shin) for s, d in concat_zero_shapes]
        for z in dev_zeros:
            z.block_until_ready()
        t0 = time.perf_counter()
        out_arrs = sharded(*dev_in, *dev_zeros)
        for o in out_arrs:
            o.block_until_ready()
        t1 = time.perf_counter()
        if run == 0:
            continue  # warmup (compile)
        dt = (t1 - t0) * 1e9
        best = dt if best is None else min(best, dt)
    results = [
        {name: np.asarray(out_arrs[i]).reshape(n_cores, *out_avals[i].shape)[cix]
         for i, name in enumerate(out_names)}
        for cix in range(n_cores)
    ]
    return results, best


def _prep_pjrt(nc, in_maps):
    """Build the sharded callable + device inputs; return a timed-call closure."""
    import time
    import jax
    import jax.numpy as jnp
    from jax.sharding import Mesh, PartitionSpec, NamedSharding
    from jax.experimental.shard_map import shard_map
    import concourse.mybir as mybir
    from concourse import bass2jax

    bass2jax.install_neuronx_cc_hook()
    n_cores = len(in_maps)
    partition_name = nc.partition_id_tensor.name if nc.partition_id_tensor else None
    in_names, out_names, out_avals, zero_outs = [], [], [], []
    for alloc in nc.m.functions[0].allocations:
        if not isinstance(alloc, mybir.MemoryLocationSet):
            continue
        name = alloc.memorylocations[0].name
        if alloc.kind == "ExternalInput":
            if name != partition_name:
                in_names.append(name)
        elif alloc.kind == "ExternalOutput":
            shape = tuple(alloc.tensor_shape)
            dtype = mybir.dt.np(alloc.dtype)
            out_names.append(name)
            out_avals.append(jax.core.ShapedArray(shape, dtype))
            zero_outs.append(np.zeros(shape, dtype))
    n_params = len(in_names)
    n_outs = len(out_avals)
    in_names_all = in_names + out_names + ([partition_name] if partition_name else [])
    donate = tuple(range(n_params, n_params + n_outs))

    def _body(*args):
        operands = list(args)
        if partition_name is not None:
            operands.append(bass2jax.partition_id_tensor())
        outs = bass2jax._bass_exec_p.bind(
            *operands, out_avals=tuple(out_avals), in_names=tuple(in_names_all),
            out_names=tuple(out_names), lowering_input_output_aliases=(),
            sim_require_finite=True, sim_require_nnan=True, nc=nc)
        return tuple(outs)

    devices = jax.devices()[:n_cores]
    mesh = Mesh(np.asarray(devices), ("core",))
    sharded = jax.jit(
        shard_map(_body, mesh=mesh,
                  in_specs=(PartitionSpec("core"),) * (n_params + n_outs),
                  out_specs=(PartitionSpec("core"),) * n_outs, check_rep=False),
        donate_argnums=donate, keep_unused=True)
    concat_in = [np.concatenate([np.asarray(in_maps[cix][name]) for cix in range(n_cores)], axis=0)
                 for name in in_names]
    zshapes = [((n_cores * z.shape[0],) + z.shape[1:], z.dtype) for z in zero_outs]
    shin = NamedSharding(mesh, PartitionSpec("core"))
    dev_in = [jax.device_put(x, shin) for x in concat_in]

    def call_timed():
        dev_zeros = [jax.device_put(jnp.zeros(sh, d), shin) for sh, d in zshapes]
        for z in dev_zeros:
            z.block_until_ready()
        t0 = time.perf_counter()
        outs = sharded(*dev_in, *dev_zeros)
        for o in outs:
            o.block_until_ready()
        t1 = time.perf_counter()
        return (t1 - t0) * 1e9, outs

    def results_of(outs):
        return [{name: np.asarray(outs[i]).reshape(n_cores, *out_avals[i].shape)[cix]
                 for i, name in enumerate(out_names)} for cix in range(n_cores)]

    return call_timed, results_of


def measure_pair(nc1, ncR, in_maps, iters, reps=8):
    """Interleaved differential timing of single vs repeated builds."""
    call1, res_of = _prep_pjrt(nc1, in_maps)
    callR, _ = _prep_pjrt(ncR, in_maps)
    call1()  # warmup/compile
    callR()
    t1s, tRs = [], []
    outs = None
    for _ in range(reps):
        t1, outs = call1()
        tR, _ = callR()
        t1s.append(t1)
        tRs.append(tR)
    T = (min(tRs) - min(t1s)) / (iters - 1)
    return res_of(outs), T, min(t1s), min(tRs)


def measure_hw_time(input_data, scales, angles, biases, indices_in,
                    iters=16, reps=6):
    """Estimate per-execution HW time by comparing a single-shot build with an
    on-device For_i(iters) build, both timed in the same session:
        T = (minwall(looped) - minwall(single)) / (iters - 1)
    Returns (output_from_single_run, T_ns, minwall_single_ns, minwall_loop_ns)."""
    input_data = np.ascontiguousarray(np.asarray(input_data, np.float32))
    c = _build_constants(angles, biases, indices_in, scales)
    in_maps = [{"xin": np.ascontiguousarray(input_data[:, i * BL:(i + 1) * BL])}
               for i in range(N_CORES)]
    nc1 = _build_bass(c)
    ncR = _build_bass(c, repeat=iters)
    res1, T, t1, tR = measure_pair(nc1, ncR, in_maps, iters, reps=max(reps, 8))
    out = np.concatenate([r["out"] for r in res1], axis=1).astype(np.float32)
    return out, T, t1, tR


def kernel(input_data, scales, angles, biases, indices_in, _profile=False):
    global LAST_EXEC_NS
    input_data = np.ascontiguousarray(np.asarray(input_data, np.float32))
    c = _build_constants(angles, biases, indices_in, scales)
    nc = _build_bass(c)
    in_maps = [{"xin": np.ascontiguousarray(input_data[:, i * BL:(i + 1) * BL])}
               for i in range(N_CORES)]
    if _profile:
        results, best_ns = _time_pjrt(nc, in_maps, n_runs=12)
        LAST_EXEC_NS = int(best_ns)
    else:
        from concourse import bass_utils
        res = bass_utils.run_bass_kernel_spmd(
            nc, in_maps, core_ids=list(range(N_CORES)), trace=False,
        )
        results = res.results
        LAST_EXEC_NS = res.exec_time_ns
    out = np.concatenate([r["out"] for r in results], axis=1)
    return out.astype(np.float32)



# revision 4
# speedup vs baseline: 1.4199x; 1.4199x over previous
"""Trainium2 Bass kernel for nn_ButterflyNetwork (self-contained).

Strategy:
- Pure data parallelism: batch 4096 -> 8 cores x 512 columns, identical program.
- All rotations composed on the host into block-diagonal 16x16 matrices; the
  in-rotation for non-activated rows is folded THROUGH the out-rotation into a
  single 128x128 matrix C per tile (y_nonact never materializes). Input scaling
  is folded into the first-consumer matrix columns.
- Canonical data rows live in a per-core DRAM arena [5120, 512] fp16 (row-major,
  1KB rows). Per module: one indirect-DMA gather of 1024 rows -> SBUF x-tiles,
  matmuls (fp16 weights, fp32 PSUM), smoothed-ReLU on ACT+DVE, z rows
  (live-only) indirect-scattered back, activation rows stored contiguously.
- Output = module 7's activation rows, cast fp16->fp32 on the final DMA.
"""
import numpy as np

# ---- problem constants (hardcoded per contract) ----
COLB = 16
IN_W = 1024
OUT_W = 512
DEPTH = 8
IN_L = 4
OUT_L = 4
ACT = 8
BLOCKS = 64
CURV = 1.0
GROW = BLOCKS * ACT
TOTAL = IN_W + DEPTH * GROW  # 5120
BATCH = 4096
N_CORES = 8
BL = BATCH // N_CORES  # 512
W = BLOCKS * COLB  # 1024
NTILES = 8
NBANKS = 4

LAST_EXEC_NS = None  # set when profiling enabled


# ---------------------------------------------------------------- host math
def _rotate(x, ang, stride):
    W_, B = x.shape
    xr = x.reshape(W_ // (2 * stride), 2, stride, B)
    a = ang.reshape(W_ // (2 * stride), stride)[:, :, None]
    cth, sth = np.cos(a), np.sin(a)
    lo, hi = xr[:, 0], xr[:, 1]
    return np.stack([cth * lo + sth * hi, -sth * lo + cth * hi], axis=1).reshape(W_, B)


def _module_rot_matrices(ang):
    I = np.eye(W)
    Min = I.copy()
    for l in range(IN_L):
        Min = _rotate(Min, ang[l], 2 ** (l % 4))
    Mout = I.copy()
    for l in range(OUT_L):
        Mout = _rotate(Mout, ang[IN_L + l], 2 ** ((IN_L + l) % 4))
    Min_b = np.stack([Min[16 * b:16 * b + 16, 16 * b:16 * b + 16] for b in range(BLOCKS)])
    Mout_b = np.stack([Mout[16 * b:16 * b + 16, 16 * b:16 * b + 16] for b in range(BLOCKS)])
    return Min_b, Mout_b


class _Consts:
    pass


def _build_constants(angles, biases, indices_in, scales):
    angles = np.asarray(angles, np.float64)
    biases = np.asarray(biases, np.float64)
    scales = np.asarray(scales, np.float64)
    idx = np.asarray(indices_in, np.int64)

    c = _Consts()
    c.Min, c.Mout = [], []
    for j in range(DEPTH):
        Min_b, Mout_b = _module_rot_matrices(angles[j])
        c.Min.append(Min_b)
        c.Mout.append(Mout_b)

    read_by = [set(idx[i].tolist()) for i in range(DEPTH)]
    c.z_live = []
    for j in range(DEPTH):
        if j == DEPTH - 1:
            c.z_live.append(np.zeros(W, bool))
            continue
        live = np.zeros(W, bool)
        for r in range(W):
            row = idx[j][r]
            for i in range(j + 1, DEPTH):
                if row in read_by[i]:
                    live[r] = True
                    break
        c.z_live.append(live)

    # consumer module of each z row (or -1 if dead)
    z_consumer = np.full((DEPTH, W), -1, np.int64)
    for j in range(DEPTH - 1):
        for r in range(W):
            row = int(idx[j][r])
            for i in range(j + 1, DEPTH):
                if row in read_by[i]:
                    z_consumer[j, r] = i
                    break
    c.z_order, c.z_nlive, c.z_nhot = [], [], []
    for j in range(DEPTH):
        orders, nlives, nhots = [], [], []
        for T in range(NTILES):
            loc = np.arange(128)
            cons = z_consumer[j, 128 * T + loc]
            hot = loc[cons == j + 1]
            cold = loc[cons > j + 1]
            dead = loc[cons == -1]
            orders.append(np.concatenate([hot, cold, dead]))
            nlives.append(int(len(hot) + len(cold)))
            nhots.append(int(len(hot)))
        c.z_order.append(orders)
        c.z_nlive.append(nlives)
        c.z_nhot.append(nhots)

    first_read = {}
    for j in range(DEPTH):
        for row in idx[j]:
            r = int(row)
            if r not in first_read:
                first_read[r] = j

    def src_factor(row, j):
        return scales[row] if (row < IN_W and first_read.get(int(row)) == j) else 1.0

    # factor per (module, tile, k): vectorized fold
    fac = np.ones((DEPTH, NTILES, 128))
    for j in range(DEPTH):
        for T in range(NTILES):
            for k in range(128):
                fac[j, T, k] = src_factor(idx[j][128 * T + k], j)

    c.W_act = np.zeros((DEPTH, NBANKS, 2, 128, 64))
    for j in range(DEPTH):
        for a in range(NBANKS):
            for h in range(2):
                T = 2 * a + h
                for m in range(64):
                    r_act = 128 * a + 64 * h + m
                    b = r_act // ACT
                    pos = r_act % ACT
                    bloc = b - 8 * T
                    ks = 16 * bloc + np.arange(16)
                    c.W_act[j, a, h, ks, m] = c.Min[j][b][pos, :] * fac[j, T, ks]

    c.C = np.zeros((DEPTH, NTILES, 128, 128))
    c.D = np.zeros((DEPTH, NTILES, 64, 128))
    for j in range(DEPTH - 1):
        Min_b, Mout_b = c.Min[j], c.Mout[j]
        # per block: composed nonact transform [16 out, 16 in]
        comp = np.einsum("bpk,bki->bpi", Mout_b[:, :, ACT:], Min_b[:, ACT:, :])
        for T in range(NTILES):
            order = c.z_order[j][T]
            for m_idx in range(128):
                r = 128 * T + order[m_idx]
                b = r // 16
                pos = r % 16
                bloc = b - 8 * T
                ks = 16 * bloc + np.arange(16)
                c.C[j, T, ks, m_idx] = comp[b][pos, :] * fac[j, T, ks]
                c.D[j, T, 8 * bloc + np.arange(ACT), m_idx] = Mout_b[b][pos, :ACT]

    c.bias = biases.reshape(DEPTH, NBANKS, 128)

    # consumer-ordered arena: slot s = 1024*j + position-in-idx_j. For each
    # produced row (init row / z row / act row) find its consuming module's slot.
    OOB = 1 << 20
    pos_in = [dict() for _ in range(DEPTH)]
    for j in range(DEPTH):
        for p, row in enumerate(idx[j]):
            pos_in[j][int(row)] = p

    def consumer_slot(row, after):
        row = int(row)
        for i in range(after, DEPTH):
            p = pos_in[i].get(row)
            if p is not None:
                return 1024 * i + p
        return OOB

    # init scatter offsets: input tile T row p (row id 128T+p)
    c.ioff = np.full((NTILES, 128), OOB, np.int32)
    for T in range(NTILES):
        for p in range(128):
            c.ioff[T, p] = consumer_slot(128 * T + p, 0)
    # z scatter offsets (live-first order; only [0:nlive] used)
    c.soff = np.full((DEPTH, NTILES, 128), OOB, np.int32)
    for j in range(DEPTH - 1):
        for T in range(NTILES):
            for m, rl in enumerate(c.z_order[j][T]):
                r = 128 * T + int(rl)
                if c.z_live[j][r]:
                    c.soff[j, T, m] = consumer_slot(idx[j][r], j + 1)
    # act scatter offsets: bank a row q = act row 128a+q = arena row IN_W+512j+128a+q
    c.aoff = np.full((DEPTH - 1, NBANKS, 128), OOB, np.int32)
    for j in range(DEPTH - 1):
        for a in range(NBANKS):
            for q in range(128):
                c.aoff[j, a, q] = consumer_slot(IN_W + GROW * j + 128 * a + q, j + 1)
    return c


# ------------------------------------------------- walrus sync-wait workaround
def _split_sync_waits(nc, limit=1):
    """This container's walrus build rejects >1 semaphore wait per instruction
    ("Too many sync wait commands"). Move excess waits onto NoOps placed just
    before the instruction on the same engine queue — the sequencer stalls at
    each NoOp's wait, so ordering semantics are identical."""
    import concourse.mybir as mybir

    seq = [0]
    for f in nc.m.functions:
        for bb in f.blocks:
            insts = bb.instructions
            newlist = []
            changed = False
            for inst in insts:
                si = getattr(inst, "sync_info", None)
                waits = list(si.on_wait) if si is not None else []
                if len(waits) > limit:
                    changed = True
                    for w in waits[:-limit]:
                        nop = mybir.InstNoOp(
                            name=f"waitsplit-{seq[0]}", ins=[], outs=[])
                        seq[0] += 1
                        nop.engine = inst.engine
                        nop.sync_info = mybir.SyncInfo(on_wait=[w], on_update=[])
                        newlist.append(nop)
                    inst.sync_info = mybir.SyncInfo(
                        on_wait=waits[-limit:], on_update=list(si.on_update))
                newlist.append(inst)
            if changed:
                bb.instructions = newlist


def _strip_scatter_dma_waits(nc, scatter_names):
    """All arena scatters write disjoint slots (each consumer slot is written
    exactly once), so scatter->scatter WAW and scatter-vs-gather WAR deps that
    Tile emits conservatively (dynamic APs) are false. True input deps of the
    scatters (offset tables, source tiles) are routed through compute-engine
    sems, so it is safe to drop DMA-lane (DMASW*/DMAHW*) waits on them."""
    import concourse.mybir as mybir

    for f in nc.m.functions:
        for bb in f.blocks:
            for inst in bb.instructions:
                if inst.name not in scatter_names:
                    continue
                si = getattr(inst, "sync_info", None)
                if si is None:
                    continue
                keep = [w for w in si.on_wait
                        if not (w.ant_name or "").startswith(("DMASW", "DMAHW"))]
                if len(keep) != len(si.on_wait):
                    inst.sync_info = mybir.SyncInfo(
                        on_wait=keep, on_update=list(si.on_update))


# ---------------------------------------------------------------- bass build
def _build_bass(c, repeat=1, knobs=None):
    import concourse.bass as bass
    import concourse.mybir as mybir
    import concourse.tile as tile
    from contextlib import ExitStack

    knobs = knobs or {}
    kn_nq = knobs.get("nq", 4)
    kn_actsq = knobs.get("actsq", "dve")
    kn_zsplit = knobs.get("zsplit", True)
    kn_scatter = knobs.get("scatter", "indirect")
    kn_actpath = knobs.get("actpath", True)
    kn_zmm = knobs.get("zmm", True)
    f16, f32, i32 = mybir.dt.float16, mybir.dt.float32, mybir.dt.int32
    AF = mybir.ActivationFunctionType
    OP = mybir.AluOpType

    nc = bass.Bass(trn_type="TRN2", num_swdge_queues=max(1, kn_nq))
    xin = nc.dram_tensor("xin", [IN_W, BL], f32, kind="ExternalInput")
    out = nc.dram_tensor("out", [OUT_W, BL], f32, kind="ExternalOutput")

    # inline constants
    wact_np = np.zeros((128, DEPTH * NBANKS * 2 * 64), np.float16)
    for j in range(DEPTH):
        for a in range(NBANKS):
            for h in range(2):
                col = ((j * NBANKS + a) * 2 + h) * 64
                wact_np[:, col:col + 64] = c.W_act[j, a, h].astype(np.float16)
    cmat_np = np.zeros((128, (DEPTH - 1) * NTILES * 128), np.float16)
    dmat_np = np.zeros((128, (DEPTH - 1) * NTILES * 128), np.float16)
    for j in range(DEPTH - 1):
        for T in range(NTILES):
            col = (j * NTILES + T) * 128
            cmat_np[:, col:col + 128] = c.C[j, T].astype(np.float16)
            po = 64 * (T % 2)
            dmat_np[po:po + 64, col:col + 128] = c.D[j, T].astype(np.float16)
    # last column of bias_np = 0.25 constant (Sqrt bias)
    bias_np = np.zeros((128, DEPTH * NBANKS + 1), np.float32)
    bias_np[:, DEPTH * NBANKS] = 0.25
    hbias_np = np.zeros((128, DEPTH * NBANKS), np.float32)
    for j in range(DEPTH):
        for a in range(NBANKS):
            bias_np[:, j * NBANKS + a] = c.bias[j, a].astype(np.float32)
            hbias_np[:, j * NBANKS + a] = (0.5 * c.bias[j, a]).astype(np.float32)
    soff_np = np.zeros((128, DEPTH * NTILES), np.int32)
    soffc_np = np.zeros((128, DEPTH * NTILES), np.int32)
    aoff_np = np.zeros((128, (DEPTH - 1) * NBANKS), np.int32)
    ioff_np = np.zeros((128, NTILES), np.int32)
    for j in range(DEPTH - 1):
        for T in range(NTILES):
            soff_np[:, j * NTILES + T] = c.soff[j, T]
            nh, nlv = c.z_nhot[j][T], c.z_nlive[j][T]
            soffc_np[0:nlv - nh, j * NTILES + T] = c.soff[j, T][nh:nlv]
        for a in range(NBANKS):
            aoff_np[:, j * NBANKS + a] = c.aoff[j, a]
    for T in range(NTILES):
        ioff_np[:, T] = c.ioff[T]

    wact_t = nc.inline_tensor(wact_np, name="wact")
    cmat_t = nc.inline_tensor(cmat_np, name="cmat")
    dmat_t = nc.inline_tensor(dmat_np, name="dmat")
    bias_t = nc.inline_tensor(bias_np, name="biast")
    hbias_t = nc.inline_tensor(hbias_np, name="hbiast")
    soff_t = nc.inline_tensor(soff_np, name="sofft")
    soffc_t = nc.inline_tensor(soffc_np, name="soffct")
    aoff_t = nc.inline_tensor(aoff_np, name="aofft")
    ioff_t = nc.inline_tensor(ioff_np, name="iofft")

    with tile.TileContext(nc) as tc, ExitStack() as ctx:
        const = ctx.enter_context(tc.tile_pool(name="const", bufs=1))
        xpool = ctx.enter_context(tc.tile_pool(name="x", bufs=2))
        apool = ctx.enter_context(tc.tile_pool(name="actp", bufs=2))
        zpool = ctx.enter_context(tc.tile_pool(name="z", bufs=2))
        pspool = ctx.enter_context(tc.tile_pool(name="ps", bufs=8, space="PSUM"))
        dram = ctx.enter_context(tc.tile_pool(name="dram", bufs=1, space="DRAM"))

        wact_sb = const.tile([128, wact_np.shape[1]], f16, tag="wact")
        cmat_sb = const.tile([128, cmat_np.shape[1]], f16, tag="cmat")
        dmat_sb = const.tile([128, dmat_np.shape[1]], f16, tag="dmat")
        bias_sb = const.tile([128, bias_np.shape[1]], f32, tag="bias")
        hbias_sb = const.tile([128, hbias_np.shape[1]], f32, tag="hbias")
        soff_sb = const.tile([128, soff_np.shape[1]], i32, tag="soff")
        soffc_sb = const.tile([128, soffc_np.shape[1]], i32, tag="soffc")
        aoff_sb = const.tile([128, aoff_np.shape[1]], i32, tag="aoff")
        ioff_sb = const.tile([128, ioff_np.shape[1]], i32, tag="ioff")
        soff_ld = const.tile([128, soff_np.shape[1]], i32, tag="soff_ld")
        soffc_ld = const.tile([128, soffc_np.shape[1]], i32, tag="soffc_ld")
        aoff_ld = const.tile([128, aoff_np.shape[1]], i32, tag="aoff_ld")
        ioff_ld = const.tile([128, ioff_np.shape[1]], i32, tag="ioff_ld")
        for sb_t, dr_t in [(bias_sb, bias_t), (hbias_sb, hbias_t),
                           (soff_ld, soff_t), (soffc_ld, soffc_t),
                           (aoff_ld, aoff_t), (ioff_ld, ioff_t)]:
            nc.sync.dma_start(out=sb_t[:], in_=dr_t[:])

        def load_mod_consts(jm):
            col = jm * NBANKS * 2 * 64
            nc.sync.dma_start(out=wact_sb[:, col:col + 512],
                              in_=wact_t[:, col:col + 512])
            if jm < DEPTH - 1:
                col = jm * NTILES * 128
                nc.sync.dma_start(out=cmat_sb[:, col:col + 1024],
                                  in_=cmat_t[:, col:col + 1024])
                nc.sync.dma_start(out=dmat_sb[:, col:col + 1024],
                                  in_=dmat_t[:, col:col + 1024])

        load_mod_consts(0)
        load_mod_consts(1)
        # offset tables rerouted through DVE so scatters' true deps are
        # compute-engine sems (see _strip_scatter_dma_waits)
        nc.vector.tensor_copy(out=soff_sb[:], in_=soff_ld[:])
        nc.vector.tensor_copy(out=soffc_sb[:], in_=soffc_ld[:])
        nc.vector.tensor_copy(out=aoff_sb[:], in_=aoff_ld[:])
        nc.vector.tensor_copy(out=ioff_sb[:], in_=ioff_ld[:])

        arena = dram.tile([NTILES * 1024, BL], f16, tag="arena")
        ARENA_MAX = NTILES * 1024 - 1
        bc_reg = nc.gpsimd.to_reg(ARENA_MAX)  # shared bounds-check register
        scatter_names = set()
        _qrr = [0]

        class _QPatch:
            def __init__(self, qname):
                self.qname = qname

            def __enter__(self):
                self._orig = mybir.InstDMACopy
                qname, orig = self.qname, self._orig

                def wrapper(*a, **kw):
                    kw["queue"] = qname
                    return orig(*a, **kw)

                mybir.InstDMACopy = wrapper

            def __exit__(self, *e):
                mybir.InstDMACopy = self._orig

        def _indirect_scatter(offs_ap, src_ap):
            q = _qrr[0] % kn_nq
            _qrr[0] += 1
            qname = f"qPoolDynamic{q}" if q else "qPoolDynamic"
            with _QPatch(qname):
                bi = nc.gpsimd.indirect_dma_start(
                    out=arena[:],
                    out_offset=bass.IndirectOffsetOnAxis(ap=offs_ap, axis=0),
                    in_=src_ap, in_offset=None,
                    bounds_check=bc_reg, oob_is_err=False,
                )
            scatter_names.add(bi.ins.name)
            return bi

        # init: xin (f32) -> cast f16 -> indirect scatter to consumer slots
        for T in range(NTILES):
            stage32 = xpool.tile([128, BL], f32, tag="init32")
            nc.sync.dma_start(out=stage32[:], in_=xin[128 * T:128 * T + 128, :])
            stage16 = xpool.tile([128, BL], f16, tag=f"init16_{T % 4}")
            nc.vector.tensor_copy(out=stage16[:], in_=stage32[:])
            _indirect_scatter(ioff_sb[:, T:T + 1], stage16[:])

        for _rep in range(repeat):
          deferred_cold = []
          for j in range(DEPTH):
              if _rep == 0 and 1 <= j < DEPTH - 1:
                  load_mod_consts(j + 1)
            # ---- gather x tiles
              xs = []
              for T in range(NTILES):
                  xt = xpool.tile([128, BL], f16, tag=f"x{T}")
                  base_r = 1024 * j + 128 * T
                  eng = nc.sync if T % 2 == 0 else nc.scalar
                  eng.dma_start(out=xt[:], in_=arena[base_r:base_r + 128, :])
                  xs.append(xt[:])

              # ---- previous module's cold scatters (off the hot chain: their
              # conservative WAR-vs-gather deps are stripped later)
              for offs_ap, src_ap in deferred_cold:
                  _indirect_scatter(offs_ap, src_ap)
              deferred_cold = []
              # ---- act banks + activation
              aos = []
              for a in range(NBANKS):
                  ps = pspool.tile([128, BL], f32, tag="ps", space="PSUM")
                  for h in range(2):
                      wcol = ((j * NBANKS + a) * 2 + h) * 64
                      nc.tensor.matmul(
                          out=ps[64 * h:64 * h + 64, :],
                          lhsT=wact_sb[:, wcol:wcol + 64],
                          rhs=xs[2 * a + h],
                          start=True, stop=True,
                          tile_position=(0, 64 * h),
                      )
                  bcol = j * NBANKS + a
                  if not kn_actpath:
                      ao = apool.tile([128, BL], f16, tag=f"ao{a}")
                      nc.vector.tensor_copy(out=ao[:], in_=ps[:])
                      aos.append(ao)
                      continue
                  # u = 0.5*(y+b); sq = u^2 (DVE, fp16); v = sqrt(sq + 0.25) = 0.5*sqrt(pre^2+1)
                  u = apool.tile([128, BL], f16, tag="u")
                  nc.vector.tensor_scalar(out=u[:], in0=ps[:], scalar1=0.5,
                                          scalar2=hbias_sb[:, bcol:bcol + 1],
                                          op0=OP.mult, op1=OP.add)
                  sq = apool.tile([128, BL], f16, tag="sq")
                  v = apool.tile([128, BL], f16, tag="v")
                  qcol = DEPTH * NBANKS
                  if kn_actsq == "dve":
                      nc.vector.tensor_tensor(out=sq[:], in0=u[:], in1=u[:], op=OP.mult)
                      nc.scalar.activation(out=v[:], in_=sq[:], func=AF.Sqrt,
                                           bias=bias_sb[:, qcol:qcol + 1], scale=1.0)
                  else:
                      nc.scalar.activation(out=sq[:], in_=ps[:], func=AF.Square,
                                           bias=bias_sb[:, bcol:bcol + 1], scale=1.0)
                      nc.scalar.activation(out=v[:], in_=sq[:], func=AF.Sqrt,
                                           bias=bias_sb[:, qcol:qcol + 1], scale=0.25)
                  ao = apool.tile([128, BL], f16, tag=f"ao{a}")
                  nc.vector.tensor_tensor(out=ao[:], in0=u[:], in1=v[:], op=OP.add)
                  aos.append(ao)

              if j == DEPTH - 1:
                  for a in range(NBANKS):
                      nc.gpsimd.dma_start(out=out[128 * a:128 * a + 128, :], in_=aos[a][:])
                  break

              # ---- z tiles: C @ x + D @ act_out, evac live rows, scatter
              for T in range(NTILES):
                  nlv = c.z_nlive[j][T]
                  if nlv == 0:
                      continue
                  col = (j * NTILES + T) * 128
                  po = 64 * (T % 2)
                  ps = pspool.tile([128, BL], f32, tag="ps", space="PSUM")
                  if kn_zmm:
                      nc.tensor.matmul(out=ps[:], lhsT=cmat_sb[:, col:col + 128],
                                       rhs=xs[T], start=True, stop=False)
                      nc.tensor.matmul(out=ps[:], lhsT=dmat_sb[po:po + 64, col:col + 128],
                                       rhs=aos[T // 2][po:po + 64, :],
                                       start=False, stop=True, tile_position=(po, 0))
                  else:
                      nc.tensor.matmul(out=ps[:], lhsT=cmat_sb[:, col:col + 128],
                                       rhs=xs[T], start=True, stop=True)
                  zq = zpool.tile([128, BL], f16, tag=f"z{T}")
                  if T in (0, 2, 4):
                      nc.vector.tensor_copy(out=zq[0:nlv, :], in_=ps[0:nlv, :])
                  else:
                      nc.scalar.copy(out=zq[0:nlv, :], in_=ps[0:nlv, :])
                  scol = j * NTILES + T
                  nh = c.z_nhot[j][T]
                  if kn_scatter != "none":
                      if not kn_zsplit:
                          _indirect_scatter(soff_sb[0:nlv, scol:scol + 1], zq[0:nlv, :])
                      else:
                          if nh > 0:
                              _indirect_scatter(soff_sb[0:nh, scol:scol + 1], zq[0:nh, :])
                          if nlv > nh:
                              deferred_cold.append(
                                  (soffc_sb[0:nlv - nh, scol:scol + 1], zq[nh:nlv, :]))
              # ---- act rows -> consumer slots
              if kn_scatter != "none":
                  for a in range(NBANKS):
                      acol = j * NBANKS + a
                      _indirect_scatter(aoff_sb[:, acol:acol + 1], aos[a][:])
    _strip_scatter_dma_waits(nc, scatter_names)
    _split_sync_waits(nc)
    return nc


# ---------------------------------------------------------------- entry point
def _time_pjrt(nc, in_maps, n_runs):
    """Replicate bass2jax.run_bass_via_pjrt's multi-core path, with a timing
    loop over executions (inputs pre-uploaded; donated zero outputs re-uploaded
    outside the timed region). Returns (results, min_wall_ns_per_exec)."""
    import time
    import jax
    import jax.numpy as jnp
    from jax.sharding import Mesh, PartitionSpec
    from jax.experimental.shard_map import shard_map
    import concourse.mybir as mybir
    from concourse import bass2jax

    bass2jax.install_neuronx_cc_hook()
    n_cores = len(in_maps)
    partition_name = nc.partition_id_tensor.name if nc.partition_id_tensor else None
    in_names, out_names, out_avals, zero_outs = [], [], [], []
    for alloc in nc.m.functions[0].allocations:
        if not isinstance(alloc, mybir.MemoryLocationSet):
            continue
        name = alloc.memorylocations[0].name
        if alloc.kind == "ExternalInput":
            if name != partition_name:
                in_names.append(name)
        elif alloc.kind == "ExternalOutput":
            shape = tuple(alloc.tensor_shape)
            dtype = mybir.dt.np(alloc.dtype)
            out_names.append(name)
            out_avals.append(jax.core.ShapedArray(shape, dtype))
            zero_outs.append(np.zeros(shape, dtype))
    n_params = len(in_names)
    n_outs = len(out_avals)
    in_names_all = in_names + out_names + ([partition_name] if partition_name else [])
    donate = tuple(range(n_params, n_params + n_outs))

    def _body(*args):
        operands = list(args)
        if partition_name is not None:
            operands.append(bass2jax.partition_id_tensor())
        outs = bass2jax._bass_exec_p.bind(
            *operands,
            out_avals=tuple(out_avals),
            in_names=tuple(in_names_all),
            out_names=tuple(out_names),
            lowering_input_output_aliases=(),
            sim_require_finite=True,
            sim_require_nnan=True,
            nc=nc,
        )
        return tuple(outs)

    devices = jax.devices()[:n_cores]
    mesh = Mesh(np.asarray(devices), ("core",))
    sharded = jax.jit(
        shard_map(_body, mesh=mesh,
                  in_specs=(PartitionSpec("core"),) * (n_params + n_outs),
                  out_specs=(PartitionSpec("core"),) * n_outs, check_rep=False),
        donate_argnums=donate, keep_unused=True,
    )
    concat_in = [
        np.concatenate([np.asarray(in_maps[c][name]) for c in range(n_cores)], axis=0)
        for name in in_names
    ]
    concat_zero_shapes = [((n_cores * z.shape[0],) + z.shape[1:], z.dtype)
                          for z in zero_outs]
    from jax.sharding import NamedSharding
    shin = NamedSharding(mesh, PartitionSpec("core"))
    dev_in = [jax.device_put(x, shin) for x in concat_in]

    best = None
    out_arrs = None
    for run in range(max(1, n_runs) + 1):
        dev_zeros = [jax.device_put(jnp.zeros(s, d), shin) for s, d in concat_zero_shapes]
        for z in dev_zeros:
            z.block_until_ready()
        t0 = time.perf_counter()
        out_arrs = sharded(*dev_in, *dev_zeros)
        for o in out_arrs:
            o.block_until_ready()
        t1 = time.perf_counter()
        if run == 0:
            continue  # warmup (compile)
        dt = (t1 - t0) * 1e9
        best = dt if best is None else min(best, dt)
    results = [
        {name: np.asarray(out_arrs[i]).reshape(n_cores, *out_avals[i].shape)[cix]
         for i, name in enumerate(out_names)}
        for cix in range(n_cores)
    ]
    return results, best


def _prep_pjrt(nc, in_maps):
    """Build the sharded callable + device inputs; return a timed-call closure."""
    import time
    import jax
    import jax.numpy as jnp
    from jax.sharding import Mesh, PartitionSpec, NamedSharding
    from jax.experimental.shard_map import shard_map
    import concourse.mybir as mybir
    from concourse import bass2jax

    bass2jax.install_neuronx_cc_hook()
    n_cores = len(in_maps)
    partition_name = nc.partition_id_tensor.name if nc.partition_id_tensor else None
    in_names, out_names, out_avals, zero_outs = [], [], [], []
    for alloc in nc.m.functions[0].allocations:
        if not isinstance(alloc, mybir.MemoryLocationSet):
            continue
        name = alloc.memorylocations[0].name
        if alloc.kind == "ExternalInput":
            if name != partition_name:
                in_names.append(name)
        elif alloc.kind == "ExternalOutput":
            shape = tuple(alloc.tensor_shape)
            dtype = mybir.dt.np(alloc.dtype)
            out_names.append(name)
            out_avals.append(jax.core.ShapedArray(shape, dtype))
            zero_outs.append(np.zeros(shape, dtype))
    n_params = len(in_names)
    n_outs = len(out_avals)
    in_names_all = in_names + out_names + ([partition_name] if partition_name else [])
    donate = tuple(range(n_params, n_params + n_outs))

    def _body(*args):
        operands = list(args)
        if partition_name is not None:
            operands.append(bass2jax.partition_id_tensor())
        outs = bass2jax._bass_exec_p.bind(
            *operands, out_avals=tuple(out_avals), in_names=tuple(in_names_all),
            out_names=tuple(out_names), lowering_input_output_aliases=(),
            sim_require_finite=True, sim_require_nnan=True, nc=nc)
        return tuple(outs)

    devices = jax.devices()[:n_cores]
    mesh = Mesh(np.asarray(devices), ("core",))
    sharded = jax.jit(
        shard_map(_body, mesh=mesh,
                  in_specs=(PartitionSpec("core"),) * (n_params + n_outs),
                  out_specs=(PartitionSpec("core"),) * n_outs, check_rep=False),
        donate_argnums=donate, keep_unused=True)
    concat_in = [np.concatenate([np.asarray(in_maps[cix][name]) for cix in range(n_cores)], axis=0)
                 for name in in_names]
    zshapes = [((n_cores * z.shape[0],) + z.shape[1:], z.dtype) for z in zero_outs]
    shin = NamedSharding(mesh, PartitionSpec("core"))
    dev_in = [jax.device_put(x, shin) for x in concat_in]

    def call_timed():
        dev_zeros = [jax.device_put(jnp.zeros(sh, d), shin) for sh, d in zshapes]
        for z in dev_zeros:
            z.block_until_ready()
        t0 = time.perf_counter()
        outs = sharded(*dev_in, *dev_zeros)
        for o in outs:
            o.block_until_ready()
        t1 = time.perf_counter()
        return (t1 - t0) * 1e9, outs

    def results_of(outs):
        return [{name: np.asarray(outs[i]).reshape(n_cores, *out_avals[i].shape)[cix]
                 for i, name in enumerate(out_names)} for cix in range(n_cores)]

    return call_timed, results_of


def measure_pair(nc1, ncR, in_maps, iters, reps=8):
    """Interleaved differential timing of single vs repeated builds."""
    call1, res_of = _prep_pjrt(nc1, in_maps)
    callR, _ = _prep_pjrt(ncR, in_maps)
    call1()  # warmup/compile
    callR()
    t1s, tRs = [], []
    outs = None
    for _ in range(reps):
        t1, outs = call1()
        tR, _ = callR()
        t1s.append(t1)
        tRs.append(tR)
    T = (min(tRs) - min(t1s)) / (iters - 1)
    return res_of(outs), T, min(t1s), min(tRs)


def measure_hw_time(input_data, scales, angles, biases, indices_in,
                    iters=16, reps=6):
    """Estimate per-execution HW time by comparing a single-shot build with an
    on-device For_i(iters) build, both timed in the same session:
        T = (minwall(looped) - minwall(single)) / (iters - 1)
    Returns (output_from_single_run, T_ns, minwall_single_ns, minwall_loop_ns)."""
    input_data = np.ascontiguousarray(np.asarray(input_data, np.float32))
    c = _build_constants(angles, biases, indices_in, scales)
    in_maps = [{"xin": np.ascontiguousarray(input_data[:, i * BL:(i + 1) * BL])}
               for i in range(N_CORES)]
    nc1 = _build_bass(c)
    ncR = _build_bass(c, repeat=iters)
    res1, T, t1, tR = measure_pair(nc1, ncR, in_maps, iters, reps=max(reps, 8))
    out = np.concatenate([r["out"] for r in res1], axis=1).astype(np.float32)
    return out, T, t1, tR


def kernel(input_data, scales, angles, biases, indices_in, _profile=False):
    global LAST_EXEC_NS
    input_data = np.ascontiguousarray(np.asarray(input_data, np.float32))
    c = _build_constants(angles, biases, indices_in, scales)
    nc = _build_bass(c)
    in_maps = [{"xin": np.ascontiguousarray(input_data[:, i * BL:(i + 1) * BL])}
               for i in range(N_CORES)]
    if _profile:
        results, best_ns = _time_pjrt(nc, in_maps, n_runs=12)
        LAST_EXEC_NS = int(best_ns)
    else:
        from concourse import bass_utils
        res = bass_utils.run_bass_kernel_spmd(
            nc, in_maps, core_ids=list(range(N_CORES)), trace=False,
        )
        results = res.results
        LAST_EXEC_NS = res.exec_time_ns
    out = np.concatenate([r["out"] for r in results], axis=1)
    return out.astype(np.float32)

